# revision 1
# baseline (speedup 1.0000x reference)
"""Trainium2 Bass kernel for nn_AF_2 (dense per-branch MLP gating).

Math (reference):
    s = t.sum(axis=1)                                  # (B, D)
    h = relu(BN1(einsum('nid,bd->bni', W1, s) + b1))   # (B, NB, I)
    y = BN2(einsum('ndi,bni->bnd', W2, h) + b2)        # (B, NB, D)
    out = (sigmoid(y) * t).sum(axis=1) * 3             # (B, D)

Strategy:
  - 8-way data parallel over B (512 rows/core), zero collectives.
  - Host folds the (inference-mode) BatchNorms into W/b, pre-transposes
    t to (DIM, NB, B) and weights into SBUF-ready layouts, casts matmul
    operands to bf16.
  - On device everything lives in a transposed (d-on-partition,
    batch-on-free) layout so that both bias adds fuse into ScalarE
    activations (per-partition bias).
  - s^T is accumulated with TensorE identity-matmuls into PSUM (exact f32).
  - GEMM1/GEMM2 are per-branch bf16 matmuls accumulated in PSUM f32.
  - Final r^T += 3*w^T (.) t^T on VectorE; output transposed on host.
"""

import os
import sys

import numpy as np

sys.path.insert(0, "/opt/trn_rl_repo")

import ml_dtypes

B, NB, DIM, R = 4096, 64, 512, 4
INTER = DIM // R  # 128
EPS = 1e-5
NCORES = 8
BS = B // NCORES  # 512 rows per core
NDC = DIM // 128  # 4 d-chunks
NG = 8            # branches per t DMA group
NGRP = NB // NG   # 8 groups
NSLOT = 6         # t ring-buffer slots

_CACHE = {}


def _build_nc():
    import concourse.bass as bass
    import concourse.mybir as mybir

    bf16 = mybir.dt.bfloat16
    f32 = mybir.dt.float32
    AF = mybir.ActivationFunctionType
    OP = mybir.AluOpType

    nc = bass.Bass("TRN2", debug=False, target_bir_lowering=False)

    t_ext = nc.declare_dram_parameter("t", [DIM, NB, BS], bf16, isOutput=False)
    w1_ext = nc.declare_dram_parameter("w1t", [128, NB, NDC, INTER], bf16, isOutput=False)
    w2_ext = nc.declare_dram_parameter("w2t", [INTER, NB, NDC, 128], bf16, isOutput=False)
    b1_ext = nc.declare_dram_parameter("b1", [INTER, NB], f32, isOutput=False)
    b2_ext = nc.declare_dram_parameter("b2", [128, NB, NDC], f32, isOutput=False)
    id_ext = nc.declare_dram_parameter("ident", [128, 128], bf16, isOutput=False)
    out_ext = nc.declare_dram_parameter("out", [NDC, 128, BS], f32, isOutput=True)

    R_PE = (0, 1)    # dc accumulated on PE (PSUM, exact)
    R_POOL = (2, 3)  # dc accumulated on GPSIMD (f32 adds in SBUF)

    from contextlib import ExitStack
    ctx = ExitStack()
    with ctx:
        s_w = ctx.enter_context(nc.semaphore("s_w"))
        s_out = ctx.enter_context(nc.semaphore("s_out"))
        s_slot = [ctx.enter_context(nc.semaphore(f"s_slot{i}")) for i in range(NSLOT)]
        s_pe = ctx.enter_context(nc.semaphore("s_pe"))
        s_act = ctx.enter_context(nc.semaphore("s_act"))
        s_dve = ctx.enter_context(nc.semaphore("s_dve"))
        s_pool = ctx.enter_context(nc.semaphore("s_pool"))
        ident_sb = ctx.enter_context(nc.sbuf_tensor("ident_sb", [128, 128], bf16))
        w1_sb = ctx.enter_context(nc.sbuf_tensor("w1_sb", [128, NB, NDC, INTER], bf16))
        w2_sb = ctx.enter_context(nc.sbuf_tensor("w2_sb", [INTER, NB, NDC, 128], bf16))
        b1_sb = ctx.enter_context(nc.sbuf_tensor("b1_sb", [INTER, NB], f32))
        b2_sb = ctx.enter_context(nc.sbuf_tensor("b2_sb", [128, NB, NDC], f32))
        tt_sb = ctx.enter_context(nc.sbuf_tensor("tt_sb", [128, NSLOT, NG, BS], bf16))
        s_sb = ctx.enter_context(nc.sbuf_tensor("s_sb", [128, NDC, BS], bf16))
        h_sb = ctx.enter_context(nc.sbuf_tensor("h_sb", [INTER, 2, BS], bf16))
        w_sb = ctx.enter_context(nc.sbuf_tensor("w_sb", [128, NDC, 2, BS], bf16))
        p_sb = ctx.enter_context(nc.sbuf_tensor("p_sb", [128, NDC, 2, BS], bf16))
        racc_sb = ctx.enter_context(nc.sbuf_tensor("racc_sb", [128, NDC, BS], f32))
        # PSUM: 4 banks phase-A s-accum; in phase B two of them hold r for
        # dc in R_PE. ps_y 4 banks, ps_h 2 banks.
        ps_s = [ctx.enter_context(nc.psum_tensor(f"ps_s{dc}", [128, BS], f32)) for dc in range(NDC)]
        ps_y = [ctx.enter_context(nc.psum_tensor(f"ps_y{i}", [128, BS], f32)) for i in range(2)]
        ps_h = [ctx.enter_context(nc.psum_tensor(f"ps_h{i}", [INTER, BS], f32)) for i in range(2)]
        # alias: phase-B y banks = ps_y[0], ps_y[1], ps_s[2], ps_s[3]?  No:
        # ps_s[2],ps_s[3] freed after s-copy; use them as y banks for dc 2,3.
        y_bank = {0: ps_y[0], 1: ps_y[1], 2: ps_s[2], 3: ps_s[3]}
        r_bank = {0: ps_s[0], 1: ps_s[1]}

        # ---------- static schedule ----------
        n_tdma = 2 * NGRP * NDC
        tslot_done = [16 * (k // NSLOT + 1) for k in range(n_tdma)]

        def tk(phase, g, dc):
            return phase * NGRP * NDC + g * NDC + dc

        NWG = NGRP  # weight groups (one per NG branches)
        pe_groupA_done = [None] * (NGRP * NDC)
        h_ready = [None] * NB
        y_ready = [[None] * NDC for _ in range(NB)]
        racc_done_pe = [[None] * NDC for _ in range(NB)]
        relu_done = [None] * NB
        sig_done = [[None] * NDC for _ in range(NB)]
        scopy_done = [None] * NDC
        stt_done = [[None] * NDC for _ in range(NB)]
        pool_done = [[None] * NDC for _ in range(NB)]
        rcopy_done = [None] * NDC

        pe_i = 0
        for k in range(NGRP * NDC):
            pe_i += 1
            pe_groupA_done[k] = pe_i
        pe_i += 1
        h_ready[0] = pe_i
        for n in range(NB):
            if n + 1 < NB:
                pe_i += 1
                h_ready[n + 1] = pe_i
            for dc in range(NDC):
                pe_i += 1
                y_ready[n][dc] = pe_i
            if n > 0:
                for dc in R_PE:
                    pe_i += 1
                    racc_done_pe[n - 1][dc] = pe_i
        for dc in R_PE:
            pe_i += 1
            racc_done_pe[NB - 1][dc] = pe_i

        act_i = 1
        relu_done[0] = 1
        for n in range(NB):
            if n + 1 < NB:
                act_i += 1
                relu_done[n + 1] = act_i
            for dc in range(NDC):
                act_i += 1
                sig_done[n][dc] = act_i

        dve_i = 0
        for dc in range(NDC):
            dve_i += 1
            scopy_done[dc] = dve_i
        for n in range(NB):
            for dc in range(NDC):
                dve_i += 1
                stt_done[n][dc] = dve_i
        for dc in R_PE:
            dve_i += 1
            rcopy_done[dc] = dve_i

        pool_i = 0
        for n in range(NB):
            for dc in R_POOL:
                pool_i += 1
                pool_done[n][dc] = pool_i

        with nc.Block() as block:

            # ================= SP: all DMAs =================
            @block.sync
            def _(sp):
                sp.dma_start(out=ident_sb[:, :], in_=id_ext[:, :]).then_inc(s_w, 16)
                sp.dma_start(out=b1_sb[:, :], in_=b1_ext[:, :]).then_inc(s_w, 16)
                sp.dma_start(out=b2_sb[:, :, :], in_=b2_ext[:, :, :]).then_inc(s_w, 16)
                # phase A t-stream
                for g in range(NGRP):
                    for dc in range(NDC):
                        k = tk(0, g, dc)
                        slot = k % NSLOT
                        if k >= NSLOT:
                            sp.wait_ge(s_pe, pe_groupA_done[k - NSLOT])
                        sp.dma_start(
                            out=tt_sb[:, slot, :, :],
                            in_=t_ext[dc * 128:(dc + 1) * 128, g * NG:(g + 1) * NG, :],
                        ).then_inc(s_slot[slot], 16)
                # first weight group, then interleave remaining ones with tB
                sp.dma_start(out=w1_sb[:, 0:NG, :, :], in_=w1_ext[:, 0:NG, :, :]).then_inc(s_w, 16)
                sp.dma_start(out=w2_sb[:, 0:NG, :, :], in_=w2_ext[:, 0:NG, :, :]).then_inc(s_w, 16)
                for g in range(NGRP):
                    for dc in range(NDC):
                        k = tk(1, g, dc)
                        slot = k % NSLOT
                        pk = k - NSLOT
                        if pk < NGRP * NDC:
                            sp.wait_ge(s_pe, pe_groupA_done[pk])
                        else:
                            m = pk - NGRP * NDC
                            pg, pdc = divmod(m, NDC)
                            pn = pg * NG + NG - 1
                            sp.wait_ge(s_dve, stt_done[pn][pdc])
                        sp.dma_start(
                            out=tt_sb[:, slot, :, :],
                            in_=t_ext[dc * 128:(dc + 1) * 128, g * NG:(g + 1) * NG, :],
                        ).then_inc(s_slot[slot], 16)
                    if g + 1 < NGRP:
                        a, b = (g + 1) * NG, (g + 2) * NG
                        sp.wait_ge(s_w, 16 * (3 + 2 * (g + 1)))  # order weight groups
                        sp.dma_start(out=w1_sb[:, a:b, :, :], in_=w1_ext[:, a:b, :, :]).then_inc(s_w, 16)
                        sp.dma_start(out=w2_sb[:, a:b, :, :], in_=w2_ext[:, a:b, :, :]).then_inc(s_w, 16)
                for dc in range(NDC):
                    if dc in R_PE:
                        sp.wait_ge(s_dve, rcopy_done[dc])
                    else:
                        sp.wait_ge(s_pool, pool_done[NB - 1][dc])
                    sp.dma_start(out=out_ext[dc, :, :], in_=racc_sb[:, dc, :]).then_inc(s_out, 16)
                sp.wait_ge(s_out, 16 * NDC)

            # w group g complete when s_w >= 16 * (3 + 2*(g+1))
            def wg_done(g):
                return 16 * (3 + 2 * (g + 1))

            # ================= PE =================
            @block.tensor
            def _(pe):
                pe.wait_ge(s_w, 16 * 3)  # ident+biases
                for g in range(NGRP):
                    for dc in range(NDC):
                        k = tk(0, g, dc)
                        slot = k % NSLOT
                        pe.wait_ge(s_slot[slot], tslot_done[k])
                        for j in range(NG):
                            mm = pe.matmul(
                                ps_s[dc][:, :], lhsT=ident_sb[:, :],
                                rhs=tt_sb[:, slot, j, :],
                                start=(g == 0 and j == 0), stop=(g == NGRP - 1 and j == NG - 1),
                            )
                        mm.then_inc(s_pe, 1)
                # prologue: G1(0)
                pe.wait_ge(s_dve, scopy_done[NDC - 1])
                pe.wait_ge(s_w, wg_done(0))
                for dc in range(NDC):
                    mm = pe.matmul(
                        ps_h[0][:, :], lhsT=w1_sb[:, 0, dc, :], rhs=s_sb[:, dc, :],
                        start=(dc == 0), stop=(dc == NDC - 1),
                    )
                mm.then_inc(s_pe, 1)
                for n in range(NB):
                    # G1(n+1) hoisted: h always ready one branch ahead
                    if n + 1 < NB:
                        if (n + 1) % NG == 0:
                            pe.wait_ge(s_w, wg_done((n + 1) // NG))
                        for dc in range(NDC):
                            mm = pe.matmul(
                                ps_h[(n + 1) % 2][:, :], lhsT=w1_sb[:, n + 1, dc, :], rhs=s_sb[:, dc, :],
                                start=(dc == 0), stop=(dc == NDC - 1),
                            )
                        mm.then_inc(s_pe, 1)
                    pe.wait_ge(s_act, relu_done[n])
                    for dc in range(NDC):
                        if n > 0:
                            pe.wait_ge(s_act, sig_done[n - 1][dc])
                        elif dc >= 2:
                            pe.wait_ge(s_dve, scopy_done[dc])
                        pe.matmul(
                            y_bank[dc][:, :], lhsT=w2_sb[:, n, dc, :], rhs=h_sb[:, n % 2, :],
                            start=True, stop=True,
                        ).then_inc(s_pe, 1)
                    if n > 0:
                        for dc in R_PE:
                            pe.wait_ge(s_dve, stt_done[n - 1][dc])
                            pe.matmul(
                                r_bank[dc][:, :], lhsT=ident_sb[:, :], rhs=p_sb[:, dc, (n - 1) % 2, :],
                                start=(n - 1 == 0), stop=False,
                            ).then_inc(s_pe, 1)
                # trailing racc for n = NB-1
                for dc in R_PE:
                    pe.wait_ge(s_dve, stt_done[NB - 1][dc])
                    pe.matmul(
                        r_bank[dc][:, :], lhsT=ident_sb[:, :], rhs=p_sb[:, dc, (NB - 1) % 2, :],
                        start=False, stop=True,
                    ).then_inc(s_pe, 1)

            # ================= ACT =================
            @block.scalar
            def _(act):
                act.wait_ge(s_w, 16 * 3)
                act.wait_ge(s_pe, h_ready[0])
                act.activation(
                    h_sb[:, 0, :], ps_h[0][:, :], AF.Relu,
                    bias=b1_sb[:, 0:1], scale=1.0,
                ).then_inc(s_act, 1)
                for n in range(NB):
                    if n + 1 < NB:
                        # relu one branch ahead; fills the y_ready bubble
                        act.wait_ge(s_pe, h_ready[n + 1])
                        if n >= 1:
                            act.wait_ge(s_pe, y_ready[n - 1][NDC - 1])  # h WAR
                        act.activation(
                            h_sb[:, (n + 1) % 2, :], ps_h[(n + 1) % 2][:, :], AF.Relu,
                            bias=b1_sb[:, n + 1:n + 2], scale=1.0,
                        ).then_inc(s_act, 1)
                    if n >= 2:
                        act.wait_ge(s_dve, stt_done[n - 2][NDC - 1])
                    for dc in range(NDC):
                        act.wait_ge(s_pe, y_ready[n][dc])
                        act.activation(
                            w_sb[:, dc, n % 2, :], y_bank[dc][:, :], AF.Sigmoid,
                            bias=b2_sb[:, n, dc:dc + 1], scale=1.0,
                        ).then_inc(s_act, 1)

            # ================= DVE =================
            @block.vector
            def _(dve):
                dve.wait_ge(s_pe, pe_groupA_done[NGRP * NDC - 1])
                for dc in range(NDC):
                    dve.tensor_copy(s_sb[:, dc, :], ps_s[dc][:, :]).then_inc(s_dve, 1)
                for n in range(NB):
                    g, j = divmod(n, NG)
                    for dc in range(NDC):
                        if j == 0:
                            k = tk(1, g, dc)
                            dve.wait_ge(s_slot[k % NSLOT], tslot_done[k])
                        dve.wait_ge(s_act, sig_done[n][dc])
                        if n >= 2:
                            if dc in R_PE:
                                dve.wait_ge(s_pe, racc_done_pe[n - 2][dc])
                            else:
                                dve.wait_ge(s_pool, pool_done[n - 2][dc])
                        slot = tk(1, g, dc) % NSLOT
                        dve.scalar_tensor_tensor(
                            p_sb[:, dc, n % 2, :], in0=w_sb[:, dc, n % 2, :], scalar=3.0,
                            in1=tt_sb[:, slot, j, :], op0=OP.mult, op1=OP.mult,
                        ).then_inc(s_dve, 1)
                for dc in R_PE:
                    dve.wait_ge(s_pe, racc_done_pe[NB - 1][dc])
                    dve.tensor_copy(racc_sb[:, dc, :], r_bank[dc][:, :]).then_inc(s_dve, 1)

            # ================= GPSIMD: r accumulation for R_POOL =================
            @block.gpsimd
            def _(pool):
                for n in range(NB):
                    for dc in R_POOL:
                        pool.wait_ge(s_dve, stt_done[n][dc])
                        if n == 0:
                            pool.tensor_copy(
                                racc_sb[:, dc, :], p_sb[:, dc, 0, :]
                            ).then_inc(s_pool, 1)
                        else:
                            pool.tensor_add(
                                racc_sb[:, dc, :], racc_sb[:, dc, :], p_sb[:, dc, n % 2, :]
                            ).then_inc(s_pool, 1)
                    pool.drain()

    return nc

def _prep(inputs):
    t = inputs["t"]
    W1, b1, g1, beta1, m1, v1 = (
        inputs["W1"], inputs["b1"], inputs["g1"], inputs["beta1"],
        inputs["m1"], inputs["v1"],
    )
    W2, b2, g2, beta2, m2, v2 = (
        inputs["W2"], inputs["b2"], inputs["g2"], inputs["beta2"],
        inputs["m2"], inputs["v2"],
    )
    a1 = g1 / np.sqrt(v1 + EPS)  # (NB, I)
    W1f = W1 * a1[:, :, None]  # (NB, I, D)
    b1f = (b1 - m1) * a1 + beta1  # (NB, I)
    a2 = g2 / np.sqrt(v2 + EPS)  # (NB, D)
    W2f = W2 * a2[:, :, None]  # (NB, D, I)
    b2f = (b2 - m2) * a2 + beta2  # (NB, D)

    bf16 = ml_dtypes.bfloat16
    # w1t[p, n, dc, i] = W1f[n, i, dc*128+p]
    w1t = np.ascontiguousarray(
        W1f.reshape(NB, INTER, NDC, 128).transpose(3, 0, 2, 1)
    ).astype(bf16)
    # w2t[i, n, dc, dd] = W2f[n, dc*128+dd, i]
    w2t = np.ascontiguousarray(
        W2f.reshape(NB, NDC, 128, INTER).transpose(3, 0, 1, 2)
    ).astype(bf16)
    # b1[i, n]
    b1_l = np.ascontiguousarray(b1f.T).astype(np.float32)
    # b2[p, n, dc]
    b2_l = np.ascontiguousarray(
        b2f.reshape(NB, NDC, 128).transpose(2, 0, 1)
    ).astype(np.float32)
    t_bf = np.ascontiguousarray(t.transpose(2, 1, 0)).astype(bf16)  # (DIM, NB, B)
    ident = np.eye(128, dtype=bf16)
    return t_bf, w1t, w2t, b1_l, b2_l, ident


def kernel(**inputs):
    from concourse.bass_utils import run_bass_kernel_spmd

    t_bf, w1t, w2t, b1_l, b2_l, ident = _prep(inputs)

    if "nc" not in _CACHE:
        _CACHE["nc"] = _build_nc()
    nc = _CACHE["nc"]

    in_maps = []
    for c in range(NCORES):
        in_maps.append({
            "t": np.ascontiguousarray(t_bf[:, :, c * BS : (c + 1) * BS]),
            "w1t": w1t, "w2t": w2t, "b1": b1_l, "b2": b2_l, "ident": ident,
        })
    res = run_bass_kernel_spmd(nc, in_maps, core_ids=list(range(NCORES)))
    outs = []
    for c in range(NCORES):
        o = res.results[c]["out"]  # (NDC, 128, BS)
        outs.append(o.reshape(DIM, BS).T)  # (BS, DIM)
    return np.concatenate(outs, axis=0).astype(np.float32)


if __name__ == "__main__":
    rng = np.random.default_rng(0)
    fake = {
        "t": rng.standard_normal((B, NB, DIM), dtype=np.float32),
        "W1": rng.standard_normal((NB, INTER, DIM), dtype=np.float32) * 0.02,
        "b1": rng.standard_normal((NB, INTER), dtype=np.float32) * 0.02,
        "g1": 1 + 0.1 * rng.standard_normal((NB, INTER), dtype=np.float32),
        "beta1": 0.1 * rng.standard_normal((NB, INTER), dtype=np.float32),
        "m1": 0.1 * rng.standard_normal((NB, INTER), dtype=np.float32),
        "v1": rng.uniform(0.5, 1.5, (NB, INTER)).astype(np.float32),
        "W2": rng.standard_normal((NB, DIM, INTER), dtype=np.float32) * 0.02,
        "b2": rng.standard_normal((NB, DIM), dtype=np.float32) * 0.02,
        "g2": 1 + 0.1 * rng.standard_normal((NB, DIM), dtype=np.float32),
        "beta2": 0.1 * rng.standard_normal((NB, DIM), dtype=np.float32),
        "m2": 0.1 * rng.standard_normal((NB, DIM), dtype=np.float32),
        "v2": rng.uniform(0.5, 1.5, (NB, DIM)).astype(np.float32),
    }
    out = kernel(**fake)
    print("kernel ran, out shape", out.shape, out.dtype)

